# revision 16
# baseline (speedup 1.0000x reference)
"""NT-Xent contrastive loss on 8 Trainium2 NeuronCores (Bass/Tile), fp8.

Strategy (no collectives; ncfw collective latency floor ~85us):
  * Host casts embT to fp8e4 [2048, 8192] (sigma=1 fits e4m3) and W*64 to
    fp8e4; b*64 stays f32.  Slab cover: core c loads the 4 column-slabs
    S_c = {c, c+1, c+2, c+4} (mod 8) of embT (8.4 MB/core).  Every slab
    pair meets on some core (Z8 difference cover), so each distinct
    1024x1024 block of the 8192x8192 similarity matrix is computed once
    globally (the diff-4 block is deduped on host: cores 0-3 win).
  * Per core: head matmul in fp8 DoubleRow (K=256/instr), interleaved
    with the previous stage's sim units for PE density -> h' = 64h in
    psum -> bias-add drain to bf16 (ACT for stage 0 when ACT is idle,
    DVE afterwards).  L2 norm: nsq' / 64 via a (1/64)-ones bf16 matmul,
    then r = exp(-0.5*ln(x)) on ACT (Exp/Ln pinned to the one activation
    table that holds both => single table load), broadcast down
    partitions with gpsimd partition_broadcast, t_on = h*r in fp8e4
    (= 8 * normalized out).
  * 5 sim blocks/core (diag + 4 pairs): two DoubleRow matmuls per
    [128,1024] psum tile, the second reusing the loaded weights
    (ldweights=False); diag killed pre-exp with an additive -1e9 mask on
    just the [128,128] sub-block holding the diagonal; ACT
    exp(0.15625*x) with fused row-sum accum writes fp8e5 exp values;
    column sums via DoubleRow ones-matmuls over mb-pair-interleaved e5
    tiles at the end.
  * DMA: W in 2 kk-half tiles + slab0 in 8 chunks first on the sync
    queue (pipelines the first head chain), slabs 1-2 as single big
    DMAs on the otherwise-idle vector queue, slab 3 big on sync.  A few
    warm-up matmuls during the DMA wait release the PE HAM clock gate.
  * pos: bf16 product of t_h slabs 0,3 + ones-matmul + r-scales (the
    1/64 ones make possim come out unscaled).  Host combine in fp64.
"""
import math
import numpy as np
import ml_dtypes

SLOTS = [(c, (c + 1) % 8, (c + 2) % 8, (c + 4) % 8) for c in range(8)]
# sim units: (stationary slot, moving slot, e5 colsum slot or None)
UNITS = [(0, 0, 4), (0, 1, 0), (0, 2, 1), (0, 3, 2), (1, 3, 3)]

_CACHE = {}


def _build():
    if "nc" in _CACHE:
        return _CACHE["nc"]
    import concourse.bacc as bacc
    import concourse.tile as tile
    import concourse.mybir as mybir

    F32 = mybir.dt.float32
    BF16 = mybir.dt.bfloat16
    E4 = mybir.dt.float8e4
    E5 = mybir.dt.float8e5
    AF = mybir.ActivationFunctionType
    ALU = mybir.AluOpType
    DR = mybir.MatmulPerfMode.DoubleRow

    nc = bacc.Bacc("TRN2", num_devices=8, debug=False)
    a_emb = nc.dram_tensor("embT8", [2048, 4096], E4, kind="ExternalInput").ap()
    a_W = nc.dram_tensor("W8", [2048, 256], E4, kind="ExternalInput").ap()
    a_b = nc.dram_tensor("b64", [256], F32, kind="ExternalInput").ap()
    a_o1 = nc.dram_tensor("onesbf", [128, 1], BF16, kind="ExternalInput").ap()
    a_o5 = nc.dram_tensor("ones5", [128, 256], E5, kind="ExternalInput").ap()
    a_mask = nc.dram_tensor("mask", [128, 128], F32, kind="ExternalInput").ap()
    o_rp = nc.dram_tensor("rowpart", [128, 40], F32, kind="ExternalOutput").ap()
    o_cp = nc.dram_tensor("colpart", [1, 5120], F32, kind="ExternalOutput").ap()
    o_ps = nc.dram_tensor("possim", [1, 1024], F32, kind="ExternalOutput").ap()

    def mm(out, lhsT, rhs, start=True, stop=True, reuse=False, dr=True):
        inst = nc.tensor.matmul(out, lhsT, rhs, start=start, stop=stop,
                                perf_mode=DR if dr else None)
        if reuse:
            inst.ins.ldweights = False
        return inst

    with tile.TileContext(nc) as tc:
        with tc.tile_pool(name="sb", bufs=1) as sb, \
             tc.tile_pool(name="wk", bufs=2) as wk, \
             tc.tile_pool(name="hp", bufs=2, space="PSUM") as hp, \
             tc.tile_pool(name="simp", bufs=2, space="PSUM") as simp, \
             tc.tile_pool(name="smp", bufs=1, space="PSUM") as smp:

            # ---- persistent tiles + prologue DMAs, critical path first:
            # W first kk-half, then stage-0 emb chunk 0, then the rest of
            # W / slab 0, then the small constants and slab 3 on the sync
            # queue; slabs 1-2 ride the vector queue (idle until the first
            # bias drain) as single big DMAs.
            t_Wh = [sb.tile([128, 2, 2, 2, 128], E4, name=f"t_W{i}")
                    for i in range(4)]
            w_src = a_W.rearrange("(kk j p) (dh f) -> p kk j dh f",
                                  kk=8, j=2, p=128, dh=2, f=128)
            nc.sync.dma_start(t_Wh[0][:], w_src[:, 0:2])

            t_e8 = [[None] * 8 for _ in range(4)]

            def load_emb_chunk(k, kk, eng):
                t = sb.tile([128, 2, 1024], E4, name=f"t_e8_{k}_{kk}")
                esrc = a_emb[256 * kk:256 * (kk + 1),
                             1024 * k:1024 * (k + 1)]
                eng.dma_start(t[:], esrc.rearrange("(j p) s -> p j s",
                                                   j=2, p=128))
                t_e8[k][kk] = t

            def load_emb_half(k, half, eng):
                # half-slab [4 kk-groups] so stage k's first chains can
                # start while the second half is still on the wire
                t = sb.tile([128, 4, 2, 1024], E4, name=f"t_e8s_{k}_{half}")
                esrc = a_emb[1024 * half:1024 * (half + 1),
                             1024 * k:1024 * (k + 1)]
                eng.dma_start(t[:], esrc.rearrange("(kk j p) s -> p kk j s",
                                                   kk=4, j=2, p=128))
                for kk in range(4):
                    t_e8[k][4 * half + kk] = t[:, kk]

            # slab 0 + W strictly first (they gate the whole pipeline and
            # the DMA phase is bandwidth-bound), W chunks dropped in at the
            # point the head chain consumes them; chunks alternate between
            # the sync and scalar rings so two DMA rings stream in
            # parallel.  Tiny constants ride gpsimd.
            rings = [nc.sync, nc.scalar]
            t_warm = sb.tile([128, 2, 512], E4, name="t_warm")
            nc.vector.memset(t_warm[:], 0)
            load_emb_chunk(0, 0, nc.scalar)
            load_emb_chunk(0, 1, nc.sync)
            nc.scalar.dma_start(t_Wh[1][:], w_src[:, 2:4])
            load_emb_chunk(0, 2, nc.scalar)
            t_tabw = sb.tile([1, 1], F32, name="t_tabw")
            nc.scalar.activation(t_tabw[:], t_warm[0:1, 0, 0:1], AF.Exp)
            load_emb_chunk(0, 3, nc.sync)
            nc.sync.dma_start(t_Wh[2][:], w_src[:, 4:6])
            load_emb_chunk(0, 4, nc.scalar)
            load_emb_chunk(0, 5, nc.sync)
            nc.scalar.dma_start(t_Wh[3][:], w_src[:, 6:8])
            load_emb_chunk(0, 6, nc.scalar)
            load_emb_chunk(0, 7, nc.sync)
            for k in range(1, 4):
                for half in range(2):
                    load_emb_half(k, half, rings[(2 * k + half) % 2])
            t_b = sb.tile([128, 2], F32, name="t_b")
            nc.gpsimd.dma_start(t_b[:], a_b.rearrange("(dh p) -> p dh",
                                                      p=128))
            t_o1 = sb.tile([128, 1], BF16, name="t_o1")
            nc.gpsimd.dma_start(t_o1[:], a_o1[:])
            t_o5 = sb.tile([128, 2, 128], E5, name="t_o5")
            nc.gpsimd.dma_start(t_o5[:], a_o5.rearrange("p (j f) -> p j f",
                                                        j=2, f=128))
            t_mask = sb.tile([128, 128], F32, name="t_mask")
            nc.gpsimd.dma_start(t_mask[:], a_mask[:])

            t_h = [sb.tile([128, 2, 1024], BF16, name=f"t_h{k}")
                   for k in range(4)]
            t_r_tiles = [None] * 4
            t_on = sb.tile([128, 2, 4, 1024], E4, name="t_on")
            t_e5 = sb.tile([128, 8, 5, 1024], E5, name="t_e5")
            rp_a = sb.tile([128, 32], F32, name="rp_a")
            rp_b = sb.tile([128, 8], F32, name="rp_b")
            cp_st = [sb.tile([1, 1024], F32, name=f"cp_st{i}")
                     for i in range(5)]
            ps_st = sb.tile([1, 1024], F32, name="ps_st")

            # ---- PE warm-up: a few throwaway matmuls over a zeroed tile
            # during the input-DMA wait release the HAM clock gate (~3.4us
            # of activity) before the real head chains arrive.
            for mb in range(1, 8):
                nc.vector.memset(t_e5[:, mb, 4, 0:128 * mb], 0)
            for i in range(4):
                Hw = hp.tile([128, 512], F32, name=f"Hw{i}", tag="H")
                mm(Hw[:], t_warm[:, :, 0:128], t_warm[:])

            def tW(kk, dh):
                return t_Wh[kk // 2][:, kk % 2, :, dh, :]

            def head_pair(k, dh, drain_act=False):
                # two row-half chains interleaved so each W[kk,dh] load is
                # shared by both matmuls (second one reuses the weights)
                Hs = [hp.tile([128, 512], F32, name=f"H{k}_{dh}_{h}",
                              tag="H") for h in range(2)]
                for kk in range(8):
                    for h in range(2):
                        mm(Hs[h][:], tW(kk, dh),
                           t_e8[k][kk][:, :, 512 * h:512 * (h + 1)],
                           start=(kk == 0), stop=(kk == 7), reuse=(h == 1))
                for h in range(2):
                    dst = t_h[k][:, dh, 512 * h:512 * (h + 1)]
                    if drain_act:
                        nc.scalar.activation(dst, Hs[h][:], AF.Identity,
                                             bias=t_b[:, dh:dh + 1])
                    else:
                        nc.vector.tensor_scalar_add(dst, Hs[h][:],
                                                    t_b[:, dh:dh + 1])

            def norm(k, split=False):
                # t_o1 holds 1/64, so nsq psum = nsq'/64 and
                # exp(-0.5*ln(x)) = 8/sqrt(nsq') -- no activation bias
                t_sq = wk.tile([128, 2, 1024], BF16, name="t_sq", tag="sq")
                nc.vector.tensor_tensor(t_sq[:], t_h[k][:], t_h[k][:],
                                        ALU.mult)
                nsq = smp.tile([1, 1024], F32, name=f"nsq{k}", tag="sm")
                nln = wk.tile([1, 1024], F32, name="nln", tag="nln")
                t_rk = sb.tile([1, 1024], F32, name=f"t_r{k}")
                t_r_tiles[k] = t_rk
                r_bc = wk.tile([128, 1024], F32, name="r_bc", tag="rbc")
                halves = (0, 1) if split else (None,)
                for nh in range(2):
                    for dh in range(2):
                        mm(nsq[0:1, 512 * nh:512 * (nh + 1)], t_o1[:],
                           t_sq[:, dh, 512 * nh:512 * (nh + 1)],
                           start=(dh == 0), stop=(dh == 1),
                           reuse=(nh + dh > 0), dr=False)
                for nh in halves:
                    s = np.s_[:] if nh is None else \
                        np.s_[512 * nh:512 * (nh + 1)]
                    nc.scalar.activation(nln[0:1, s], nsq[0:1, s], AF.Ln)
                    nc.scalar.activation(t_rk[0:1, s], nln[0:1, s], AF.Exp,
                                         scale=-0.5)
                    nc.gpsimd.partition_broadcast(r_bc[:, s], t_rk[0:1, s])
                    for dh in range(2):
                        nc.vector.tensor_tensor(t_on[:, dh, k, s],
                                                t_h[k][:, dh, s],
                                                r_bc[:, s], ALU.mult)

            def unit(u, a, b, e5slot, mb):
                ps = simp.tile([128, 1024], F32, name="ps", tag="ps")
                lo = 128 * mb if a == b else 0
                # diag blocks are symmetric: compute only columns >= 128*mb
                # (upper triangle at tile granularity); the lower-triangle
                # row sums come back as column sums of the upper part
                segs = [(s0, s1) for s0, s1 in ((lo, 512), (512, 1024))
                        if s1 > max(s0, lo)]
                for si, (s0, s1) in enumerate(segs):
                    s0 = max(s0, lo)
                    mm(ps[:, s0:s1],
                       t_on[:, :, a, 128 * mb:128 * (mb + 1)],
                       t_on[:, :, b, s0:s1], reuse=(si == 1))
                if a == b:
                    # kill the diagonal: it lives in the [128,128]
                    # sub-block at column offset 128*mb
                    nc.vector.tensor_tensor(
                        ps[:, 128 * mb:128 * (mb + 1)],
                        ps[:, 128 * mb:128 * (mb + 1)], t_mask[:], ALU.add)
                dest = t_e5[:, mb, e5slot, lo:1024]
                acc = rp_a[:, u * 8 + mb:u * 8 + mb + 1] if u < 4 else \
                    rp_b[:, mb:mb + 1]
                nc.scalar.activation(dest, ps[:, lo:1024], AF.Exp,
                                     scale=0.15625, accum_out=acc)

            def colsums(ci):
                # column sums for e5 slot ci (DoubleRow over mb pairs)
                for nh in range(2):
                    cs = hp.tile([128, 512], F32, name=f"cs{ci}_{nh}",
                                 tag="H")
                    for jj in range(4):
                        mm(cs[:], t_o5[:],
                           t_e5[:, 2 * jj:2 * jj + 2, ci,
                                512 * nh:512 * (nh + 1)],
                           start=(jj == 0), stop=(jj == 3), reuse=(jj > 0))
                    nc.vector.tensor_copy(
                        cp_st[ci][0:1, 512 * nh:512 * (nh + 1)], cs[0:1, :])
                nc.sync.dma_start(o_cp[0:1, 1024 * ci:1024 * (ci + 1)],
                                  cp_st[ci][:])

            def emit_pos():
                # pos: bf16 product of t_h slabs 0 and 3 + ones-matmul +
                # r-scales (1/64 ones make possim come out unscaled)
                t_pp = wk.tile([128, 2, 1024], BF16, name="t_pp", tag="sq")
                nc.vector.tensor_tensor(t_pp[:], t_h[0][:], t_h[3][:],
                                        ALU.mult)
                pr = smp.tile([1, 1024], F32, name="rawdot", tag="sm")
                for nh in range(2):
                    for dh in range(2):
                        mm(pr[0:1, 512 * nh:512 * (nh + 1)], t_o1[:],
                           t_pp[:, dh, 512 * nh:512 * (nh + 1)],
                           start=(dh == 0), stop=(dh == 1),
                           reuse=(nh + dh > 0), dr=False)
                tmp = wk.tile([1, 1024], F32, name="ptmp", tag="nln")
                nc.vector.tensor_tensor(tmp[:], pr[:], t_r_tiles[0][:],
                                        ALU.mult)
                nc.vector.tensor_tensor(ps_st[:], tmp[:], t_r_tiles[3][:],
                                        ALU.mult)
                nc.sync.dma_start(o_ps, ps_st[:])

            def emit_unit(u, mb):
                unit(u, *UNITS[u][:2], UNITS[u][2], mb)

            head_pair(0, 0, drain_act=True)
            head_pair(0, 1, drain_act=True)
            norm(0, split=True)
            for k in range(1, 4):
                pu = k - 1
                emit_unit(pu, 0)
                emit_unit(pu, 1)
                head_pair(k, 0)
                emit_unit(pu, 2)
                emit_unit(pu, 3)
                head_pair(k, 1)
                emit_unit(pu, 4)
                # norm before the last three units: its ln/exp slot into
                # the exp stream and the broadcast/multiply overlap exps
                # 5-7, so U_k's first psim is ready the moment exp 7 drains
                norm(k)
                emit_unit(pu, 5)
                emit_unit(pu, 6)
                emit_unit(pu, 7)
            emit_pos()
            for mb in range(8):
                emit_unit(3, mb)
                if mb == 1:
                    colsums(0)
                elif mb == 5:
                    colsums(1)
            # rowpart for units 0-3 is complete here; ship it early
            nc.sync.dma_start(o_rp[:, 0:32], rp_a[:])
            for mb in range(8):
                emit_unit(4, mb)
                if mb == 2:
                    colsums(2)
                elif mb == 5:
                    colsums(4)
            colsums(3)
            nc.sync.dma_start(o_rp[:, 32:40], rp_b[:])

    # Keep Exp/Ln selectable only from the single table set that holds both,
    # so the compiler never ping-pongs ACT table loads between exp-only and
    # ln-only sets (1283ns per reload).  Entries stay in place so
    # act_func_set_id indices still match act_info.json.
    import concourse.bacc as bacc_mod
    orig_get = bacc_mod.get_activation_tables

    def _pinned_tables(arch):
        tabs = orig_get(arch)
        AFT = mybir.ActivationFunctionType
        both = [k for k, v in tabs.items() if AFT.Exp in v and AFT.Ln in v]
        if not both:
            return tabs
        keep = both[0]
        out = {}
        for k, v in tabs.items():
            if k == keep:
                out[k] = v
            else:
                out[k] = {f for f in v if f not in (AFT.Exp, AFT.Ln)}
        return out

    bacc_mod.get_activation_tables = _pinned_tables
    try:
        nc.compile()
    finally:
        bacc_mod.get_activation_tables = orig_get
    _CACHE["nc"] = nc
    return nc


def _host_inputs(embedded_data, W, b):
    E4np = ml_dtypes.float8_e4m3
    E5np = ml_dtypes.float8_e5m2
    emb = np.asarray(embedded_data, dtype=np.float32)
    embT8 = np.ascontiguousarray(emb.T).astype(E4np)      # [2048, 8192]
    W8 = (np.asarray(W, dtype=np.float32) * 64.0).astype(E4np)
    b64 = (np.asarray(b, dtype=np.float32) * 64.0).astype(np.float32)
    o1 = np.full((128, 1), 1.0 / 64.0, ml_dtypes.bfloat16)
    o5 = np.ones((128, 256), E5np)
    # -1e9 on the diagonal AND the within-block lower triangle: the diag
    # unit computes upper tiles [128mb:1024] whose first 128 columns hold
    # the block-diagonal square; masking its lower half (and diagonal)
    # makes acc+colsum count every off-diagonal element exactly once
    mask = np.where(np.tri(128, dtype=bool), -1e9, 0).astype(np.float32)
    in_maps = []
    for c in range(8):
        cols = np.concatenate(
            [embT8[:, 1024 * s:1024 * (s + 1)] for s in SLOTS[c]], axis=1)
        in_maps.append({"embT8": np.ascontiguousarray(cols), "W8": W8,
                        "b64": b64, "onesbf": o1, "ones5": o5, "mask": mask})
    return in_maps


def _combine(results):
    neg = np.zeros(8192, np.float64)
    pos = np.zeros(8192, np.float64)
    for c in range(8):
        S = SLOTS[c]
        rp = results[c]["rowpart"].astype(np.float64)     # [128, 40]
        cp = results[c]["colpart"].astype(np.float64).ravel()
        ps = results[c]["possim"].astype(np.float64)
        sl = [np.s_[1024 * s:1024 * (s + 1)] for s in S]
        for u, (astat, _, _) in enumerate(UNITS):
            if u == 3 and c >= 4:
                continue                                   # diff-4 dedup
            dst = 1024 * S[astat]
            for mb in range(8):
                neg[dst + 128 * mb:dst + 128 * (mb + 1)] += rp[:, 8 * u + mb]
        neg[sl[1]] += cp[0:1024]
        neg[sl[2]] += cp[1024:2048]
        if c < 4:
            neg[sl[3]] += cp[2048:3072]
        neg[sl[3]] += cp[3072:4096]
        # diag block is computed upper-triangle only; its column sums
        # supply the missing lower-triangle row sums
        neg[sl[0]] += cp[4096:5120]
        if c < 4:
            possim = ps.ravel()
            pos[sl[0]] = possim
            pos[sl[3]] = possim
    loss = -np.mean(10.0 * pos - np.log(neg))
    return np.float32(loss)


def run(embedded_data, W, b, trace=False):
    from concourse import bass_utils
    nc = _build()
    in_maps = _host_inputs(embedded_data, W, b)
    res = bass_utils.run_bass_kernel_spmd(nc, in_maps, core_ids=list(range(8)),
                                          trace=trace)
    return _combine(res.results), res


def kernel(embedded_data, W, b):
    loss, _ = run(embedded_data, W, b, trace=False)
    return np.asarray(loss, dtype=np.float32)
